# revision 20
# baseline (speedup 1.0000x reference)
"""Trainium2 Bass kernel for nn_DDPMVAEQueryEncoder (v8).

Strategy (data-parallel over batch, 8 cores):
  * Host: band-pack rows into 4 chunks/core minimizing gather padding;
    build column-major gather grids (slot = row + 128*block), fold all
    weight-only matmuls, exact 1/sqrt(nnz), and per-step noise
    (sigma/b2/bf16-compensation folded) packed [128, T*256] with chain-A
    features on partitions 0-63 and chain-B on 64-127.
  * Device per core (512 batch rows):
      phase 1: batch-major dma_gather of bf16 table rows (padded to 128
        elems); pooling = identity-lhsT matmuls accumulating each
        [128 rows, 128 feat] block into a per-chunk PSUM accumulator
        (f32-exact, 53ns/block on PE). Then rsq scale (DVE, per-partition
        scalar), PE transpose to feature-major, and one [64,256] wec
        matmul per chunk pair for the conditioning vector c.
      phase 2: 50 ancestral DDPM steps in two half-batch chains (A=chunks
        0+1, B=2+3): ph = W1s^T xc (f32r rhs full speed), silu on ACT
        (per-step bias from tb1), eps-psum accumulation via w2a/w2b
        (bf16) + A_t*x identity (f32r) + noise (bf16, K=128 top/bottom
        selection via identity columns), x <- -C_t * pe on DVE. Chunk-2/3
        reduce bursts are woven between chain-A steps on PE.
  * Host: un-permute rows, emit [4096, 64].
"""
import sys

import numpy as np

if "/opt/trn_rl_repo" not in sys.path:
    sys.path.insert(0, "/opt/trn_rl_repo")

import ml_dtypes
import concourse.bass as bass
import concourse.mybir as mybir
import concourse.tile as tile
from concourse import bacc
from concourse.bass_utils import run_bass_kernel_spmd
from concourse.masks import make_identity

F32 = mybir.dt.float32
F32R = mybir.dt.float32r
BF16 = mybir.dt.bfloat16
I16 = mybir.dt.int16

T_STEPS = 50
D = 64
B = 4096
L = 200
V = 100000
NCORES = 8
BL = B // NCORES          # 512 rows per core
HB = BL // 2              # 256 per half-batch chain
NCHUNK = 4                # 4 chunks of 128 rows
NSEG = 4
SEG = 25000               # index range per segment
SEGR = SEG + 1            # +1 zero row


def _schedule_consts():
    steps = T_STEPS
    scale = 1000.0 / steps
    betas = np.linspace(scale * 1e-4, scale * 2e-2, steps, dtype=np.float64)
    alphas = 1.0 - betas
    acp = np.cumprod(alphas)
    acp_prev = np.append(1.0, acp[:-1])
    sqrt_recip = np.sqrt(1.0 / acp)
    sqrt_recipm1 = np.sqrt(1.0 / acp - 1.0)
    post_var = betas * (1.0 - acp_prev) / (1.0 - acp)
    post_logvar = np.log(np.append(post_var[1], post_var[1:]))
    coef1 = betas * np.sqrt(acp_prev) / (1.0 - acp)
    coef2 = (1.0 - acp_prev) * np.sqrt(alphas) / (1.0 - acp)
    f32 = lambda a: a.astype(np.float32)
    sr, srm1, plv, c1, c2 = map(f32, (sqrt_recip, sqrt_recipm1, post_logvar, coef1, coef2))
    A = (c1 * sr + c2).astype(np.float32)
    C = (c1 * srm1).astype(np.float32)
    S = np.exp(0.5 * plv).astype(np.float32)
    S[0] = 0.0
    return A, C, S


def _timestep_tables(Wt, bt, W1, b1):
    half = D // 2
    freqs = np.exp(-np.log(10000.0) * np.arange(half, dtype=np.float32) / half)
    t = np.arange(T_STEPS, dtype=np.float32)
    args = t[:, None] * freqs[None, :]
    temb = np.concatenate([np.cos(args), np.sin(args)], axis=-1).astype(np.float32)
    tt = (temb @ Wt + bt).astype(np.float32)
    return (b1 + tt @ W1).astype(np.float32)  # [50, 256]


def host_prep(inputs):
    seq = np.asarray(inputs["seq"]).astype(np.int64)
    item_emb = np.asarray(inputs["item_emb"], dtype=np.float32)
    W_enc = np.asarray(inputs["W_enc"], dtype=np.float32)
    b_enc = np.asarray(inputs["b_enc"], dtype=np.float32)
    Wt = np.asarray(inputs["Wt"], dtype=np.float32)
    bt = np.asarray(inputs["bt"], dtype=np.float32)
    Wc = np.asarray(inputs["Wc"], dtype=np.float32)
    bc = np.asarray(inputs["bc"], dtype=np.float32)
    W1 = np.asarray(inputs["W1"], dtype=np.float32)
    b1 = np.asarray(inputs["b1"], dtype=np.float32)
    W2 = np.asarray(inputs["W2"], dtype=np.float32)
    b2 = np.asarray(inputs["b2"], dtype=np.float32)
    init_noise = np.asarray(inputs["init_noise"], dtype=np.float32)
    step_noise = np.asarray(inputs["step_noise"], dtype=np.float32)

    A, C, S = _schedule_consts()

    # ---- band packing: greedy pack rows into 4 bands of 1024 minimizing
    # per-band per-seg max counts (gather padding); leanest bands first.
    bucket = seq // SEG
    counts = np.stack([(bucket == k).sum(1) for k in range(NSEG)], 1)
    mx = counts.max(1)
    idx_desc = np.argsort(-mx, kind="stable")
    bands = [[] for _ in range(NCHUNK)]
    bmax = np.zeros((NCHUNK, NSEG), np.int64)
    for r in idx_desc:
        best, bestcost = None, None
        for b in range(NCHUNK):
            if len(bands[b]) >= NCORES * 128:
                continue
            cost = np.maximum(bmax[b], counts[r]).sum() - bmax[b].sum()
            if bestcost is None or cost < bestcost:
                best, bestcost = b, cost
        bands[best].append(r)
        bmax[best] = np.maximum(bmax[best], counts[r])
    border = np.argsort(bmax.sum(1), kind="stable")    # leanest first
    order = np.concatenate([np.array(bands[b]) for b in border])
    rows = order.reshape(NCHUNK, NCORES, 128)          # [chunk, core, row]

    G = counts[order].reshape(NCHUNK, NCORES * 128, NSEG).max(1)
    G = np.maximum(G, 1).astype(np.int64)              # [chunk, 4]

    # ---- bf16 table rows padded to 128 elems: [emb | zeros]
    tbl = np.zeros((NSEG * SEGR, 128), ml_dtypes.bfloat16)
    for k in range(NSEG):
        tbl[k * SEGR: k * SEGR + SEG, :D] = \
            item_emb[k * SEG: (k + 1) * SEG].astype(ml_dtypes.bfloat16)

    # ---- column-major gather idx grids: slot = r + 128*j; idx layout
    # [16, S/16] (slot i at [i%16, i//16]) replicated to 128 partitions.
    idx16 = [[[None] * NSEG for _ in range(NCHUNK)] for _ in range(NCORES)]
    for c in range(NCHUNK):
        for n in range(NCORES):
            rs = rows[c, n]
            sq = seq[rs]
            bk = bucket[rs]
            for k in range(NSEG):
                g = int(G[c, k])
                val = np.full((128, g), SEG, np.int16)
                for p in range(128):
                    e = sq[p][bk[p] == k] - k * SEG
                    val[p, : len(e)] = e.astype(np.int16)
                flat = val.T.reshape(-1)               # slot = r + 128*j
                s_tot = 128 * g
                arr = flat.reshape(s_tot // 16, 16).T  # [16, S/16]
                idx16[n][c][k] = np.ascontiguousarray(np.tile(arr, (8, 1)))

    # ---- folded weights
    wec = (W_enc[:, :D] @ Wc).astype(np.float32)
    bec = (b_enc[:D] @ Wc + bc).astype(np.float32).reshape(D, 1)
    w1s = np.vstack([W1, W1]).astype(np.float32)           # [128, 256]
    w2a = np.ascontiguousarray(W2[:128]).astype(ml_dtypes.bfloat16)
    w2b = np.ascontiguousarray(W2[128:]).astype(ml_dtypes.bfloat16)
    TB1 = _timestep_tables(Wt, bt, W1, b1)
    tb1 = np.ascontiguousarray(
        np.concatenate([TB1[:, :128].T, TB1[:, 128:].T], axis=1))  # [128, 100]
    NBLK = (T_STEPS + 7) // 8
    tb16 = np.zeros((16, NBLK * 128), np.float32)
    for t in range(T_STEPS):
        blk, q = t // 8, t % 8
        tb16[q, blk * 128:blk * 128 + 128] = TB1[t, :128]
        tb16[8 + q, blk * 128:blk * 128 + 128] = TB1[t, 128:]
    tb16 = tb16.astype(ml_dtypes.bfloat16)
    onesel = np.zeros((16, 8 * 512), np.float32)
    for q in range(8):
        onesel[q, q * 512:q * 512 + 256] = 1.0
        onesel[8 + q, q * 512 + 256:(q + 1) * 512] = 1.0
    onesel = onesel.astype(ml_dtypes.bfloat16)

    # iax: (A_t / -C_t) I, f32 [64, 50*64]
    iax = np.zeros((D, T_STEPS * D), np.float32)
    for t in range(T_STEPS):
        iax[:, t * D:(t + 1) * D] = (A[t] / (-C[t])) * np.eye(D, dtype=np.float32)

    nnz = np.count_nonzero(seq, axis=1).astype(np.float32)
    rsq_all = (1.0 / np.sqrt(nnz)).astype(np.float32)      # [4096]

    per_core = []
    for n in range(NCORES):
        rws = rows[:, n, :].reshape(-1)                    # 512 rows, chunk order
        # noise'' feature-major: (S_t n_i - C_t b2)/(-C_t), step i -> t=49-i
        nT = (step_noise[:, rws, :].transpose(0, 2, 1) * S[::-1, None, None]
              - (C[::-1, None] * b2[None, :])[:, :, None])     # [50, 64, 512]
        nT = nT / (-C[::-1, None, None])
        nz = np.zeros((128, T_STEPS * HB), np.float32)
        for i in range(T_STEPS):
            nz[:D, i * HB:(i + 1) * HB] = nT[i, :, 0:HB]       # chain A
            nz[D:, i * HB:(i + 1) * HB] = nT[i, :, HB:BL]      # chain B
        nz = np.ascontiguousarray(nz).astype(ml_dtypes.bfloat16)
        x0T = np.ascontiguousarray(init_noise[rws].T)          # [64, 512]
        rsqc = np.ascontiguousarray(
            rsq_all[rws].reshape(NCHUNK, 128).T).astype(np.float32)  # [128, 4]
        core = dict(tbl=tbl, nz=nz, x0T=x0T, rsqc=rsqc,
                    w1s=w1s, w2a=w2a, w2b=w2b, wec=wec, bec=bec, tb1=tb1,
                    iax=iax, tb16=tb16, onesel=onesel)
        for c in range(NCHUNK):
            for k in range(NSEG):
                core[f"idx_{c}_{k}"] = idx16[n][c][k]
        per_core.append((core, rws))

    consts = dict(A=A, C=C, S=S)
    return per_core, G, consts


def build_program(G, consts, STEP_NS=2100.0, JOB_FUDGE=1.0):
    """G: [NCHUNK, NSEG] gather grid widths. Returns compiled nc."""
    A, C, S = consts["A"], consts["C"], consts["S"]
    nc = bacc.Bacc("TRN2", target_bir_lowering=False, debug=False,
                   num_devices=NCORES)

    din = lambda name, shape, dt=F32: nc.dram_tensor(
        name, shape, dt, kind="ExternalInput").ap()
    tbl_d = din("tbl", [NSEG * SEGR, 128], BF16)
    nz_d = din("nz", [128, T_STEPS * HB], BF16)
    x0T_d = din("x0T", [D, BL], F32R)
    rsqc_d = din("rsqc", [128, NCHUNK])
    w1s_d = din("w1s", [128, 256], F32R)
    w2a_d = din("w2a", [128, D], BF16)
    w2b_d = din("w2b", [128, D], BF16)
    wec_d = din("wec", [D, D], F32R)
    bec_d = din("bec", [D, 1])
    tb1_d = din("tb1", [128, 2 * T_STEPS])
    NBLK = (T_STEPS + 7) // 8
    tb16_d = din("tb16", [16, NBLK * 128], BF16)
    onesel_d = din("onesel", [16, 8 * 512], BF16)
    iax_d = din("iax", [D, T_STEPS * D], F32R)
    idx_d = {}
    for c in range(NCHUNK):
        for k in range(NSEG):
            idx_d[(c, k)] = din(f"idx_{c}_{k}", [128, 8 * int(G[c, k])], I16)
    outT_d = nc.dram_tensor("outT", [D, BL], F32, kind="ExternalOutput").ap()

    Gmax = int(G.max())

    with tile.TileContext(nc) as tc:
        with (
            tc.tile_pool(name="const", bufs=1) as constp,
            tc.tile_pool(name="gidx", bufs=1) as gidxp,
            tc.tile_pool(name="gdst", bufs=3) as gdstp,
            tc.tile_pool(name="pools", bufs=4) as poolsp,
            tc.tile_pool(name="xcp", bufs=1) as xcp,
            tc.tile_pool(name="hp", bufs=4) as hp,
            tc.tile_pool(name="ps_r", bufs=2, space="PSUM") as ps_r,
            tc.tile_pool(name="ps_t", bufs=1, space="PSUM") as ps_t,
            tc.tile_pool(name="ps_h", bufs=2, space="PSUM") as ps_h,
            tc.tile_pool(name="ps_e", bufs=2, space="PSUM") as ps_e,
        ):
            # ---- phase-early DMAs: idx for chunks 0,1 then small consts
            idx_t = {}

            def load_idx(c):
                for k in range(NSEG):
                    g = int(G[c, k])
                    it = gidxp.tile([128, 8 * Gmax], I16, name=f"it{c}{k}",
                                    tag=f"it{c}{k}")
                    nc.sync.dma_start(it[:, : 8 * g], idx_d[(c, k)][:])
                    idx_t[(c, k)] = it

            load_idx(0)
            load_idx(1)

            w1s = constp.tile([128, 256], F32R, name="w1s")
            nc.sync.dma_start(w1s[:], w1s_d[:])
            w2a = constp.tile([128, D], BF16, name="w2a")
            nc.sync.dma_start(w2a[:], w2a_d[:])
            w2b = constp.tile([128, D], BF16, name="w2b")
            nc.sync.dma_start(w2b[:], w2b_d[:])
            wec = constp.tile([D, D], F32R, name="wec")
            nc.sync.dma_start(wec[:], wec_d[:])
            bec = constp.tile([D, 1], F32, name="bec")
            nc.sync.dma_start(bec[:], bec_d[:])
            tb1 = constp.tile([128, 2 * T_STEPS], F32, name="tb1")
            nc.sync.dma_start(tb1[:], tb1_d[:])
            rsqc = constp.tile([128, NCHUNK], F32, name="rsqc")
            nc.sync.dma_start(rsqc[:], rsqc_d[:])
            xch = [xcp.tile([128, HB], F32R, name=f"xc{h}", tag=f"xc{h}")
                   for h in range(2)]
            for h in range(2):
                nc.sync.dma_start(xch[h][0:D, :],
                                  x0T_d[:, h * HB:(h + 1) * HB])

            ident = constp.tile([128, 128], F32, name="ident")
            make_identity(nc, ident[:])
            identb = constp.tile([128, 128], BF16, name="identb")
            nc.vector.tensor_copy(identb[:], ident[:])

            # ---- gather + PE-reduce machinery
            def do_gather(c, k):
                g = int(G[c, k])
                s_tot = 128 * g
                dst = gdstp.tile([128, 128 * Gmax], BF16, name="dst", tag="dst")
                nc.gpsimd.dma_gather(
                    out_ap=dst[:, :s_tot].rearrange(
                        "p (j f) -> p j f", j=g, f=128),
                    in_ap=tbl_d[k * SEGR:(k + 1) * SEGR, :],
                    idxs_ap=idx_t[(c, k)][:, : 8 * g],
                    num_idxs=s_tot,
                    num_idxs_reg=s_tot,
                    elem_size=128,
                    transpose=False,
                    single_packet=False,
                )
                return dst

            racc = {}

            def reduce_burst(c, k, dst):
                """Accumulate gather blocks into chunk-c PSUM via identity."""
                g = int(G[c, k])
                if c not in racc:
                    racc[c] = ps_r.tile([128, 128], F32, name=f"racc{c}",
                                        tag="racc")
                view = dst[:, :128 * g].rearrange("p (j f) -> p j f", j=g, f=128)
                first = (k == 0)
                last_k = (k == NSEG - 1)
                for j in range(g):
                    nc.tensor.matmul(
                        out=racc[c][:], lhsT=identb[:], rhs=view[:, j, :],
                        start=(first and j == 0),
                        stop=(last_k and j == g - 1))

            poolT2 = [constp.tile([D, HB], F32R, name=f"poolT{h}")
                      for h in range(2)]

            def pool_finish(c):
                """racc -> rsq scale (bf16) -> PE transpose -> feature-major."""
                ps = poolsp.tile([128, 128], F32, name=f"pool{c}", tag="pool")
                nc.vector.tensor_scalar(
                    out=ps[:], in0=racc[c][:], scalar1=rsqc[:, c:c + 1],
                    scalar2=None, op0=mybir.AluOpType.mult)
                pt = ps_t.tile([128, 128], F32, name="pt", tag="pt")
                nc.tensor.transpose(out=pt[:], in_=ps[:], identity=ident[:])
                h, half = c // 2, (c % 2) * 128
                nc.scalar.activation(
                    poolT2[h][:, half:half + 128], pt[0:D, :],
                    mybir.ActivationFunctionType.Identity)

            def do_pc(h):
                pc = ps_t.tile([D, HB], F32, name="pc", tag="pc")
                nc.tensor.matmul(out=pc[:], lhsT=wec[:], rhs=poolT2[h][:],
                                 start=True, stop=True)
                nc.scalar.activation(xch[h][D:128, :], pc[:],
                                     mybir.ActivationFunctionType.Identity,
                                     bias=bec[:, 0:1])

            # ---- scan step
            nzt = constp.tile([128, T_STEPS * HB], BF16, name="nzt")
            iax = constp.tile([D, T_STEPS * D], F32R, name="iax")

            def do_step(h, i):
                t = T_STEPS - 1 - i
                xc = xch[h]
                blk, q = t // 8, t % 8
                ph = ps_h.tile([128, 512], F32, name="ph", tag="ph")
                nc.tensor.matmul(out=ph[:, 0:256],
                                 lhsT=tb16[:, blk * 128:(blk + 1) * 128],
                                 rhs=onesel[:, q * 512:q * 512 + 256],
                                 start=True, stop=False)
                nc.tensor.matmul(out=ph[:, 256:512],
                                 lhsT=tb16[:, blk * 128:(blk + 1) * 128],
                                 rhs=onesel[:, q * 512 + 256:(q + 1) * 512],
                                 start=True, stop=False)
                nc.tensor.matmul(out=ph[:, 0:256], lhsT=w1s[:, 0:128],
                                 rhs=xc[:], start=False, stop=True)
                nc.tensor.matmul(out=ph[:, 256:512], lhsT=w1s[:, 128:256],
                                 rhs=xc[:], start=False, stop=True)
                h_ab = hp.tile([128, 512], BF16, name="h_ab", tag="h")
                nc.scalar.activation(h_ab[:], ph[:],
                                     mybir.ActivationFunctionType.Silu)
                pe = ps_e.tile([D, HB], F32, name="pe", tag="pe")
                nc.tensor.matmul(out=pe[:],
                                 lhsT=identb[:, h * D:(h + 1) * D],
                                 rhs=nzt[:, i * HB:(i + 1) * HB],
                                 start=True, stop=False)
                nc.tensor.matmul(out=pe[:],
                                 lhsT=iax[:, t * D:(t + 1) * D],
                                 rhs=xc[0:D, :], start=False, stop=False)
                nc.tensor.matmul(out=pe[:], lhsT=w2a[:],
                                 rhs=h_ab[:, 0:256], start=False, stop=False)
                nc.tensor.matmul(out=pe[:], lhsT=w2b[:],
                                 rhs=h_ab[:, 256:512], start=False, stop=True)
                nc.vector.tensor_scalar(
                    out=xc[0:D, :], in0=pe[:],
                    scalar1=-float(C[t]), scalar2=None,
                    op0=mybir.AluOpType.mult)

            # ---- schedule
            # chunks 0,1: gathers + eager reduce bursts, finish, wec pair 0
            for c in (0, 1):
                for k in range(NSEG):
                    dst = do_gather(c, k)
                    reduce_burst(c, k, dst)
                pool_finish(c)
            do_pc(0)

            # phase-late DMAs (needed from first scan step onward)
            tb16 = constp.tile([16, NBLK * 128], BF16, name="tb16")
            nc.sync.dma_start(tb16[:], tb16_d[:])
            onesel = constp.tile([16, 8 * 512], BF16, name="onesel")
            nc.sync.dma_start(onesel[:], onesel_d[:])
            nc.sync.dma_start(nzt[:], nz_d[:])
            nc.sync.dma_start(iax[:], iax_d[:])
            load_idx(2)
            load_idx(3)

            # chunk 2,3 gathers pipelined; reduce bursts interleaved with a
            # THROTTLED chain A (~PRE_A steps before B goes live) so most A
            # steps remain to pair with chain B after the last gather.
            ck_list = [(c, k) for c in (2, 3) for k in range(NSEG)]
            PIPE_G = 2
            PRE_A = 2
            gdsts = {}
            for j in range(PIPE_G):
                gdsts[j] = do_gather(*ck_list[j])

            ia, ib = 0, 0
            for j, (c, k) in enumerate(ck_list):
                for _ in range(PRE_A):
                    if ia < T_STEPS:
                        do_step(0, ia)
                        ia += 1
                reduce_burst(c, k, gdsts.pop(j))
                if j + PIPE_G < len(ck_list):
                    gdsts[j + PIPE_G] = do_gather(*ck_list[j + PIPE_G])
                if k == NSEG - 1:
                    pool_finish(c)
                    if c == 3:
                        do_pc(1)

            while ia < T_STEPS or ib < T_STEPS:
                if ib < T_STEPS:
                    do_step(1, ib)
                    ib += 1
                if ia < T_STEPS:
                    do_step(0, ia)
                    ia += 1

            for h in range(2):
                nc.sync.dma_start(outT_d[:, h * HB:(h + 1) * HB],
                                  xch[h][0:D, :].bitcast(F32))

    nc.compile()
    return nc


_CACHE = {}


def _get_program(G, consts):
    key = tuple(G.reshape(-1).tolist())
    if key not in _CACHE:
        _CACHE[key] = build_program(G, consts)
    return _CACHE[key]


def kernel(**inputs):
    per_core, G, consts = host_prep(inputs)
    nc = _get_program(G, consts)
    in_maps = [core for core, _ in per_core]
    res = run_bass_kernel_spmd(nc, in_maps, list(range(NCORES)))
    out = np.zeros((B, D), np.float32)
    for n in range(NCORES):
        _, rws = per_core[n]
        out[rws] = res.results[n]["outT"].T
    return out
